# revision 1
# baseline (speedup 1.0000x reference)
"""Multi-head attention (B=4, S=2048, D=1024, H=16) on 8 trn2 NeuronCores.

Sharding: core c -> (batch b = c//2, head-group hg = c%2 of 8 heads).
Each core computes q/k/v projections for its 8 heads, attention, and a
partial output projection (its heads' contribution). Host sums the two
partials per batch and adds b_O.

Per-core device pipeline:
  1. projections: qT/kT [512,2048] (head-pair stacked on partitions),
     v-hat [128, 8, 65] in natural [s,c] layout (lhsT=XvT tile, rhs=Wv)
     with a ones column appended (softmax Z falls out of the PV matmul)
  2. per (head-pair, q-half): scoresT = kT.T @ qT (two heads row-packed,
     K=64), ACT exp(scale=1/8) -> PT, PV accumulates out_unT[65, q]
  3. tail: stage psum->SBUF (frees PSUM fast), recipZ, PE K=1 broadcast
     matmul, multiply -> attn_outT [512, 2048]
  4. output projection: attn_outT.T @ Wo -> partial [2048, 1024] fp32
"""
import sys

if '/opt/trn_rl_repo' not in sys.path:
    sys.path.insert(0, '/opt/trn_rl_repo')

import ml_dtypes
import numpy as np

import concourse.bass as bass
import concourse.tile as tile
from concourse import bacc, mybir
from concourse.bass_utils import run_bass_kernel_spmd

N_CORES = 8
B, S, D = 4, 2048, 1024
H = 16
DH = 64                 # head dim
HC = 8                  # heads per core
C = HC * DH             # per-core projection width = 512
F32 = mybir.dt.float32
F32R = mybir.dt.float32r
BF16 = mybir.dt.bfloat16

NKT = S // 128          # 16 s-tiles of 128
NM = C // 128           # 4 c-tiles (head pairs)
NDK = D // 128          # 8 contraction tiles for projections
SCALE = 1.0 / np.sqrt(DH)

# dtype config for the four matmul stages (BF16 or F32R)
PROJ_DT = BF16          # q/k/v projection inputs (XT, W)
QK_DT = BF16            # qT/kT tiles (scores matmul inputs)
PV_DT = BF16            # PT + v-hat (PV matmul inputs)
OUT_DT = BF16           # attn_outT + Wo (output projection inputs)


def round_fp32r(x):
    b = np.ascontiguousarray(x, dtype=np.float32).view(np.uint32)
    b = (b + 0x800) & np.uint32(0xFFFFF000)
    return b.view(np.float32)


def prep(x, dt):
    if dt == BF16:
        return np.ascontiguousarray(x).astype(ml_dtypes.bfloat16)
    return round_fp32r(x)


def build():
    nc = bacc.Bacc("TRN2", target_bir_lowering=False, debug=False,
                   num_devices=N_CORES)
    XqT = nc.dram_tensor("XqT", [D, S], PROJ_DT, kind="ExternalInput").ap()
    XkT = nc.dram_tensor("XkT", [D, S], PROJ_DT, kind="ExternalInput").ap()
    XvT = nc.dram_tensor("XvT", [D, S], PROJ_DT, kind="ExternalInput").ap()
    Wq = nc.dram_tensor("Wq", [D, C], PROJ_DT, kind="ExternalInput").ap()
    Wk = nc.dram_tensor("Wk", [D, C], PROJ_DT, kind="ExternalInput").ap()
    Wv = nc.dram_tensor("Wv", [D, C], PROJ_DT, kind="ExternalInput").ap()
    Wo = nc.dram_tensor("Wo", [C, D], OUT_DT, kind="ExternalInput").ap()
    bq = nc.dram_tensor("bq", [C], F32, kind="ExternalInput").ap()
    bk = nc.dram_tensor("bk", [C], F32, kind="ExternalInput").ap()
    bv = nc.dram_tensor("bv", [C], F32, kind="ExternalInput").ap()
    OP = nc.dram_tensor("OP", [S, D], F32, kind="ExternalOutput").ap()

    with tile.TileContext(nc) as tc:
        _build_body(nc, tc, XqT, XkT, XvT, Wq, Wk, Wv, Wo, bq, bk, bv, OP)
    nc.compile()
    return nc


def _build_body(nc, tc, XqT, XkT, XvT, Wq, Wk, Wv, Wo, bq, bk, bv, OP):
    from contextlib import ExitStack
    with ExitStack() as stack:
        consts = stack.enter_context(tc.tile_pool(name="consts", bufs=1))
        qkp = stack.enter_context(tc.tile_pool(name="qk", bufs=2 * NM))
        vhp = stack.enter_context(tc.tile_pool(name="vh", bufs=NKT))
        aop = stack.enter_context(tc.tile_pool(name="aout", bufs=NM))

        # constants
        ones_f32 = consts.tile([128, 1], F32)
        nc.vector.memset(ones_f32, 1.0)
        ones_row = consts.tile([1, DH], F32R)
        with nc.allow_low_precision(reason="exact ones to fp32r"):
            nc.vector.tensor_copy(
                ones_row, ones_f32[0:1, :].broadcast_to((1, DH)))

        bias_t = consts.tile([128, 2 * NM], F32)
        for i, b_ in enumerate((bq, bk)):
            nc.sync.dma_start(
                out=bias_t[:, i * NM:(i + 1) * NM],
                in_=b_.rearrange("(m p) -> p m", p=128))
        bvb = consts.tile([128, C], F32)
        nc.gpsimd.dma_start(
            out=bvb,
            in_=bass.AP(tensor=bv.tensor, offset=0, ap=[[0, 128], [1, C]]))

        # ---------------- phase 1: projections ----------------
        qT = [None] * NM
        kT = [None] * NM
        vhat = [None] * NKT
        with ExitStack() as pstack:
            xtp = pstack.enter_context(tc.tile_pool(name="xt", bufs=16))
            wp = pstack.enter_context(tc.tile_pool(name="w", bufs=12))
            pjp = pstack.enter_context(
                tc.tile_pool(name="pj", bufs=3, space="PSUM"))

            for m in range(NM):
                qT[m] = qkp.tile([128, S], QK_DT, tag="qk", name=f"qTt{m}")
                kT[m] = qkp.tile([128, S], QK_DT, tag="qk", name=f"kTt{m}")

            # v in natural [s, c] layout: lhsT = XvT tile, rhs = Wv
            for half in range(2):
                xts = []
                for kt in range(NDK):
                    xt = xtp.tile([128, S // 2], PROJ_DT, tag="xt",
                                  name=f"xvt{half}_{kt}")
                    nc.sync.dma_start(
                        out=xt,
                        in_=XvT[kt * 128:(kt + 1) * 128,
                                half * (S // 2):(half + 1) * (S // 2)])
                    xts.append(xt)
                ws = []
                for kt in range(NDK):
                    w = wp.tile([128, C], PROJ_DT, tag="w",
                                name=f"wv{half}_{kt}")
                    nc.sync.dma_start(out=w, in_=Wv[kt * 128:(kt + 1) * 128, :])
                    ws.append(w)
                for stl in range(8):
                    st = half * 8 + stl
                    ps = pjp.tile([128, C], F32, tag="pj", name=f"vps{st}")
                    for kt in range(NDK):
                        nc.tensor.matmul(
                            ps,
                            xts[kt][:, stl * 128:(stl + 1) * 128],
                            ws[kt],
                            start=(kt == 0), stop=(kt == NDK - 1))
                    vh = vhp.tile([128, HC, DH + 1], PV_DT, tag="vh",
                                  name=f"vhat{st}")
                    with nc.allow_low_precision(reason="v epilogue"):
                        nc.vector.tensor_add(
                            vh[:, :, 0:DH],
                            ps.rearrange("p (h d) -> p h d", h=HC),
                            bvb.rearrange("p (h d) -> p h d", h=HC))
                        nc.vector.tensor_copy(
                            vh[:, :, DH], ones_f32.broadcast_to((128, HC)))
                    vhat[st] = vh

            def projection(XT, W, bcol, outs):
                for half in range(2):
                    xts = []
                    for kt in range(NDK):
                        xt = xtp.tile([128, S // 2], PROJ_DT, tag="xt")
                        nc.sync.dma_start(
                            out=xt,
                            in_=XT[kt * 128:(kt + 1) * 128,
                                   half * (S // 2):(half + 1) * (S // 2)])
                        xts.append(xt)
                    ws = []
                    for kt in range(NDK):
                        w = wp.tile([128, C], PROJ_DT, tag="w")
                        nc.sync.dma_start(
                            out=w, in_=W[kt * 128:(kt + 1) * 128, :])
                        ws.append(w)
                    for m in range(NM):
                        for sc in range(2):
                            ps = pjp.tile([128, 512], F32, tag="pj")
                            for kt in range(NDK):
                                nc.tensor.matmul(
                                    ps,
                                    ws[kt][:, m * 128:(m + 1) * 128],
                                    xts[kt][:, sc * 512:(sc + 1) * 512],
                                    start=(kt == 0), stop=(kt == NDK - 1))
                            s0 = half * (S // 2) + sc * 512
                            with nc.allow_low_precision(reason="proj epi"):
                                nc.scalar.activation(
                                    out=outs[m][:, s0:s0 + 512], in_=ps,
                                    func=mybir.ActivationFunctionType.Identity,
                                    bias=bias_t[:, bcol + m:bcol + m + 1],
                                    scale=1.0)

            projection(XkT, Wk, NM, kT)
            projection(XqT, Wq, 0, qT)

        # ---------------- phase 2: attention ----------------
        attn_outT = [None] * NM
        for m in range(NM):
            attn_outT[m] = aop.tile([128, S], OUT_DT, tag="aout",
                                    name=f"aoutT{m}")

        with ExitStack() as astack:
            ptp = astack.enter_context(tc.tile_pool(name="pt", bufs=8))
            stg = astack.enter_context(tc.tile_pool(name="stg", bufs=6))
            nrm = astack.enter_context(tc.tile_pool(name="nrm", bufs=6))
            wop = astack.enter_context(tc.tile_pool(name="wo", bufs=NM))
            oap = astack.enter_context(tc.tile_pool(name="oacc", bufs=32))
            sp = astack.enter_context(
                tc.tile_pool(name="sps", bufs=2, space="PSUM"))
            pvp = astack.enter_context(
                tc.tile_pool(name="pv", bufs=2, space="PSUM"))
            opp = astack.enter_context(
                tc.tile_pool(name="op", bufs=2, space="PSUM"))

            wo_tiles = []
            for m in range(NM):
                w = wop.tile([128, D], OUT_DT, tag="wo", name=f"wo{m}")
                nc.sync.dma_start(out=w, in_=Wo[m * 128:(m + 1) * 128, :])
                wo_tiles.append(w)
            out_acc = [[None] * 2 for _ in range(NKT)]

            def outproj_piece(hp, st):
                for oc in range(2):
                    ps = opp.tile([128, 512], F32, tag="op",
                                  name=f"ops{hp}_{st}_{oc}")
                    nc.tensor.matmul(
                        ps,
                        attn_outT[hp][:, st * 128:(st + 1) * 128],
                        wo_tiles[hp][:, oc * 512:(oc + 1) * 512],
                        start=True, stop=True)
                    if hp == 0:
                        oa = oap.tile([128, 512], F32, tag="oacc",
                                      name=f"oacc{st}_{oc}")
                        out_acc[st][oc] = oa
                        nc.vector.tensor_copy(oa, ps)
                    else:
                        oa = out_acc[st][oc]
                        nc.vector.tensor_add(oa, oa, ps)
                    if hp == NM - 1:
                        nc.sync.dma_start(
                            out=OP[st * 128:(st + 1) * 128,
                                   oc * 512:(oc + 1) * 512],
                            in_=oa)

            # deferred tail/outproj pieces, drained at fixed kt slots of
            # the NEXT block so slow DVE work never head-of-line blocks PE
            deferred = []

            def make_tail_pieces(hp, qc, pvA, pvB):
                q0 = qc * 512
                sts = [None, None]
                rzs = [None, None]
                bcs = [None, None]

                def stage(hh):
                    acc = pvA if hh == 0 else pvB
                    st_t = stg.tile([DH + 1, 512], F32R, tag="stg",
                                    name=f"stg{hp}_{qc}_{hh}")
                    with nc.allow_low_precision(reason="stage"):
                        nc.vector.tensor_copy(st_t, acc)
                    sts[hh] = st_t

                def recip(hh):
                    rz = nrm.tile([1, 512], F32R, tag="rz",
                                  name=f"rz{hp}_{qc}_{hh}")
                    with nc.allow_low_precision(reason="recipZ"):
                        nc.vector.reciprocal(out=rz, in_=sts[hh][DH:DH + 1, :])
                    rzs[hh] = rz

                def bcast(hh):
                    bc_ps = opp.tile([DH, 512], F32, tag="op",
                                     name=f"bcp{hp}_{qc}_{hh}")
                    nc.tensor.matmul(bc_ps, ones_row, rzs[hh],
                                     start=True, stop=True)
                    bc = nrm.tile([DH, 512], F32R, tag="bc",
                                  name=f"bc{hp}_{qc}_{hh}")
                    with nc.allow_low_precision(reason="bc"):
                        nc.vector.tensor_copy(bc, bc_ps)
                    bcs[hh] = bc

                def mul(hh):
                    dlo = hh * DH
                    with nc.allow_low_precision(reason="attn_outT"):
                        nc.vector.tensor_mul(
                            attn_outT[hp][dlo:dlo + DH, q0:q0 + 512],
                            sts[hh][0:DH, :], bcs[hh])

                # stages run NOW (free the PSUM accumulators quickly)
                stage(0)
                stage(1)
                return [
                    lambda: recip(0),
                    lambda: recip(1),
                    lambda: bcast(0),
                    lambda: bcast(1),
                    lambda: mul(0),
                    lambda: mul(1),
                    lambda: outproj_piece(hp, qc * 4 + 0),
                    lambda: outproj_piece(hp, qc * 4 + 1),
                    lambda: outproj_piece(hp, qc * 4 + 2),
                    lambda: outproj_piece(hp, qc * 4 + 3),
                ]

            # kt slots at which deferred pieces fire (10 pieces)
            SLOTS = {0: 0, 1: 1, 3: 2, 4: 3, 6: 4, 7: 5,
                     9: 6, 11: 7, 13: 8, 15: 9}

            for hp in range(NM):
                for qc in range(4):
                    q0 = qc * 512
                    pvA = pvp.tile([DH + 1, 512], F32, tag="pv",
                                   name=f"pvA{hp}_{qc}")
                    pvB = pvp.tile([DH + 1, 512], F32, tag="pv",
                                   name=f"pvB{hp}_{qc}")
                    for kt in range(NKT):
                        sps = sp.tile([128, 1024], F32, tag="sps")
                        for hh in range(2):
                            dlo = hh * DH
                            nc.tensor.matmul(
                                sps[:, hh * 512:(hh + 1) * 512],
                                kT[hp][dlo:dlo + DH,
                                       kt * 128:(kt + 1) * 128],
                                qT[hp][dlo:dlo + DH, q0:q0 + 512],
                                start=True, stop=True)
                        pt = ptp.tile([128, 1024], PV_DT, tag="pt")
                        nc.scalar.activation(
                            out=pt, in_=sps,
                            func=mybir.ActivationFunctionType.Exp,
                            scale=float(SCALE))
                        nc.tensor.matmul(
                            pvA, vhat[kt][:, 2 * hp, :], pt[:, 0:512],
                            start=(kt == 0), stop=(kt == NKT - 1))
                        nc.tensor.matmul(
                            pvB, vhat[kt][:, 2 * hp + 1, :], pt[:, 512:1024],
                            start=(kt == 0), stop=(kt == NKT - 1))
                        if kt in SLOTS and deferred:
                            deferred[SLOTS[kt]]()
                    deferred = make_tail_pieces(hp, qc, pvA, pvB)

            # flush the last block's pieces
            for piece in deferred:
                piece()


_NC_CACHE = None
_last_in_maps = None


def _get_nc():
    global _NC_CACHE
    if _NC_CACHE is None:
        _NC_CACHE = build()
    return _NC_CACHE


def kernel(Q, K, V, W_Q, b_Q, W_K, b_K, W_V, b_V, W_O, b_O):
    global _last_in_maps
    Q = np.asarray(Q, dtype=np.float32)
    K = np.asarray(K, dtype=np.float32)
    V = np.asarray(V, dtype=np.float32)
    nc = _get_nc()

    XqTs = [prep(Q[b].T, PROJ_DT) for b in range(B)]
    XkTs = [prep(K[b].T, PROJ_DT) for b in range(B)]
    XvTs = [prep(V[b].T, PROJ_DT) for b in range(B)]
    Wqs = [prep(np.asarray(W_Q)[:, hg * C:(hg + 1) * C], PROJ_DT)
           for hg in range(2)]
    Wks = [prep(np.asarray(W_K)[:, hg * C:(hg + 1) * C], PROJ_DT)
           for hg in range(2)]
    Wvs = [prep(np.asarray(W_V)[:, hg * C:(hg + 1) * C], PROJ_DT)
           for hg in range(2)]
    Wos = [prep(np.asarray(W_O)[hg * C:(hg + 1) * C, :], OUT_DT)
           for hg in range(2)]
    bqs = [np.ascontiguousarray(np.asarray(b_Q, dtype=np.float32)[hg * C:(hg + 1) * C])
           for hg in range(2)]
    bks = [np.ascontiguousarray(np.asarray(b_K, dtype=np.float32)[hg * C:(hg + 1) * C])
           for hg in range(2)]
    bvs = [np.ascontiguousarray(np.asarray(b_V, dtype=np.float32)[hg * C:(hg + 1) * C])
           for hg in range(2)]

    in_maps = []
    for c in range(N_CORES):
        b, hg = c // 2, c % 2
        in_maps.append({
            "XqT": XqTs[b], "XkT": XkTs[b], "XvT": XvTs[b],
            "Wq": Wqs[hg], "Wk": Wks[hg], "Wv": Wvs[hg], "Wo": Wos[hg],
            "bq": bqs[hg], "bk": bks[hg], "bv": bvs[hg],
        })
    _last_in_maps = in_maps
    res = run_bass_kernel_spmd(nc, in_maps, list(range(N_CORES)))
    out = np.empty((B, S, D), dtype=np.float32)
    bO = np.asarray(b_O, dtype=np.float32)
    for b in range(B):
        out[b] = res.results[2 * b]["OP"] + res.results[2 * b + 1]["OP"] + bO
    return out



# revision 5
# speedup vs baseline: 8.3853x; 8.3853x over previous
"""Multi-head attention (B=4, S=2048, D=1024, H=16) on 8 trn2 NeuronCores.

Sharding: core c -> (batch b = c//2, head-group hg = c%2 of 8 heads).
Each core computes q/k/v projections for its 8 heads, attention, and a
partial output projection (its heads' contribution). Host sums the two
partials per batch and adds b_O.

Per-core device pipeline:
  1. projections: qT/kT [512,2048] (head-pair stacked on partitions),
     v-hat [128, 8, 65] in natural [s,c] layout (lhsT=XvT tile, rhs=Wv)
     with a ones column appended (softmax Z falls out of the PV matmul)
  2. per (head-pair, q-half): scoresT = kT.T @ qT (two heads row-packed,
     K=64), ACT exp(scale=1/8) -> PT, PV accumulates out_unT[65, q]
  3. tail: stage psum->SBUF (frees PSUM fast), recipZ, PE K=1 broadcast
     matmul, multiply -> attn_outT [512, 2048]
  4. output projection: attn_outT.T @ Wo -> partial [2048, 1024] fp32
"""
import sys

if '/opt/trn_rl_repo' not in sys.path:
    sys.path.insert(0, '/opt/trn_rl_repo')

import ml_dtypes
import numpy as np

import concourse.bass as bass
import concourse.tile as tile
from concourse import bacc, mybir
from concourse.bass_utils import run_bass_kernel_spmd

N_CORES = 8
B, S, D = 4, 2048, 1024
H = 16
DH = 64                 # head dim
HC = 8                  # heads per core
C = HC * DH             # per-core projection width = 512
F32 = mybir.dt.float32
F32R = mybir.dt.float32r
BF16 = mybir.dt.bfloat16

NKT = S // 128          # 16 s-tiles of 128
NM = C // 128           # 4 c-tiles (head pairs)
NDK = D // 128          # 8 contraction tiles for projections
SCALE = 1.0 / np.sqrt(DH)

# dtype config for the four matmul stages (BF16 or F32R)
PROJ_DT = BF16          # q/k/v projection inputs (XT, W)
QK_DT = BF16            # qT/kT tiles (scores matmul inputs)
PV_DT = BF16            # PT + v-hat (PV matmul inputs)
OUT_DT = BF16           # attn_outT + Wo (output projection inputs)


def round_fp32r(x):
    b = np.ascontiguousarray(x, dtype=np.float32).view(np.uint32)
    b = (b + 0x800) & np.uint32(0xFFFFF000)
    return b.view(np.float32)


def prep(x, dt):
    if dt == BF16:
        return np.ascontiguousarray(x).astype(ml_dtypes.bfloat16)
    return round_fp32r(x)


def build(repeat=1):
    nc = bacc.Bacc("TRN2", target_bir_lowering=False, debug=False,
                   num_devices=N_CORES)
    XqT = nc.dram_tensor("XqT", [D, S], PROJ_DT, kind="ExternalInput").ap()
    XkT = nc.dram_tensor("XkT", [D, S], PROJ_DT, kind="ExternalInput").ap()
    XvT = nc.dram_tensor("XvT", [D, S], PROJ_DT, kind="ExternalInput").ap()
    Wq = nc.dram_tensor("Wq", [D, C], PROJ_DT, kind="ExternalInput").ap()
    Wk = nc.dram_tensor("Wk", [D, C], PROJ_DT, kind="ExternalInput").ap()
    Wv = nc.dram_tensor("Wv", [D, C], PROJ_DT, kind="ExternalInput").ap()
    Wo = nc.dram_tensor("Wo", [C, D], OUT_DT, kind="ExternalInput").ap()
    bq = nc.dram_tensor("bq", [C], F32, kind="ExternalInput").ap()
    bk = nc.dram_tensor("bk", [C], F32, kind="ExternalInput").ap()
    bv = nc.dram_tensor("bv", [C], F32, kind="ExternalInput").ap()
    OP = nc.dram_tensor("OP", [S, D], F32, kind="ExternalOutput").ap()

    SCR = None
    if repeat > 1:
        SCR = nc.dram_tensor("SCR", [1, 8], F32, kind="Internal").ap()
    with tile.TileContext(nc) as tc:
        token = None
        for _rep in range(repeat):
            token = _build_body(nc, tc, XqT, XkT, XvT, Wq, Wk, Wv, Wo,
                                bq, bk, bv, OP, gate=token, SCR=SCR)
    nc.compile()
    return nc


def _build_body(nc, tc, XqT, XkT, XvT, Wq, Wk, Wv, Wo, bq, bk, bv, OP,
                gate=None, SCR=None):
    from contextlib import ExitStack
    with ExitStack() as stack:
        consts = stack.enter_context(tc.tile_pool(name="consts", bufs=1))
        if gate is not None:
            nc.sync.dma_start(out=SCR, in_=gate[0:1, 0:8])
        qkp = stack.enter_context(tc.tile_pool(name="qk", bufs=2 * NM))
        vhp = stack.enter_context(tc.tile_pool(name="vh", bufs=NKT))
        aop = stack.enter_context(tc.tile_pool(name="aout", bufs=NM))

        # constants
        ones_f32 = consts.tile([128, 1], F32)
        nc.vector.memset(ones_f32, 1.0)
        ones_row = consts.tile([1, DH], F32R)
        with nc.allow_low_precision(reason="exact ones to fp32r"):
            nc.vector.tensor_copy(
                ones_row, ones_f32[0:1, :].broadcast_to((1, DH)))

        bias_t = consts.tile([128, 2 * NM], F32)
        for i, b_ in enumerate((bq, bk)):
            nc.sync.dma_start(
                out=bias_t[:, i * NM:(i + 1) * NM],
                in_=b_.rearrange("(m p) -> p m", p=128))
        bvb = consts.tile([128, C], F32)
        nc.gpsimd.dma_start(
            out=bvb,
            in_=bass.AP(tensor=bv.tensor, offset=0, ap=[[0, 128], [1, C]]))

        # ---------------- phase 1: projections ----------------
        qT = [None] * NM
        kT = [None] * NM
        vhat = [None] * NKT
        with ExitStack() as pstack:
            xtp = pstack.enter_context(tc.tile_pool(name="xt", bufs=16))
            wp = pstack.enter_context(tc.tile_pool(name="w", bufs=12))
            pjp = pstack.enter_context(
                tc.tile_pool(name="pj", bufs=3, space="PSUM"))

            for m in range(NM):
                qT[m] = qkp.tile([128, S], QK_DT, tag="qk", name=f"qTt{m}")
                kT[m] = qkp.tile([128, S], QK_DT, tag="qk", name=f"kTt{m}")

            # v in natural [s, c] layout: lhsT = XvT tile, rhs = Wv
            for half in range(2):
                xts = []
                for kt in range(NDK):
                    xt = xtp.tile([128, S // 2], PROJ_DT, tag="xt",
                                  name=f"xvt{half}_{kt}")
                    nc.sync.dma_start(
                        out=xt,
                        in_=XvT[kt * 128:(kt + 1) * 128,
                                half * (S // 2):(half + 1) * (S // 2)])
                    xts.append(xt)
                ws = []
                for kt in range(NDK):
                    w = wp.tile([128, C], PROJ_DT, tag="w",
                                name=f"wv{half}_{kt}")
                    nc.sync.dma_start(out=w, in_=Wv[kt * 128:(kt + 1) * 128, :])
                    ws.append(w)
                for stl in range(8):
                    st = half * 8 + stl
                    ps = pjp.tile([128, C], F32, tag="pj", name=f"vps{st}")
                    for kt in range(NDK):
                        nc.tensor.matmul(
                            ps,
                            xts[kt][:, stl * 128:(stl + 1) * 128],
                            ws[kt],
                            start=(kt == 0), stop=(kt == NDK - 1))
                    vh = vhp.tile([128, HC, DH + 1], PV_DT, tag="vh",
                                  name=f"vhat{st}")
                    with nc.allow_low_precision(reason="v epilogue"):
                        nc.vector.tensor_add(
                            vh[:, :, 0:DH],
                            ps.rearrange("p (h d) -> p h d", h=HC),
                            bvb.rearrange("p (h d) -> p h d", h=HC))
                        nc.vector.tensor_copy(
                            vh[:, :, DH], ones_f32.broadcast_to((128, HC)))
                    vhat[st] = vh

            def projection(XT, W, bcol, outs):
                for half in range(2):
                    xts = []
                    for kt in range(NDK):
                        xt = xtp.tile([128, S // 2], PROJ_DT, tag="xt")
                        nc.sync.dma_start(
                            out=xt,
                            in_=XT[kt * 128:(kt + 1) * 128,
                                   half * (S // 2):(half + 1) * (S // 2)])
                        xts.append(xt)
                    ws = []
                    for kt in range(NDK):
                        w = wp.tile([128, C], PROJ_DT, tag="w")
                        nc.sync.dma_start(
                            out=w, in_=W[kt * 128:(kt + 1) * 128, :])
                        ws.append(w)
                    for m in range(NM):
                        for sc in range(2):
                            ps = pjp.tile([128, 512], F32, tag="pj")
                            for kt in range(NDK):
                                nc.tensor.matmul(
                                    ps,
                                    ws[kt][:, m * 128:(m + 1) * 128],
                                    xts[kt][:, sc * 512:(sc + 1) * 512],
                                    start=(kt == 0), stop=(kt == NDK - 1))
                            s0 = half * (S // 2) + sc * 512
                            with nc.allow_low_precision(reason="proj epi"):
                                nc.scalar.activation(
                                    out=outs[m][:, s0:s0 + 512], in_=ps,
                                    func=mybir.ActivationFunctionType.Identity,
                                    bias=bias_t[:, bcol + m:bcol + m + 1],
                                    scale=1.0)

            projection(XkT, Wk, NM, kT)
            projection(XqT, Wq, 0, qT)

        # ---------------- phase 2: attention ----------------
        attn_outT = [None] * NM
        for m in range(NM):
            attn_outT[m] = aop.tile([128, S], OUT_DT, tag="aout",
                                    name=f"aoutT{m}")

        with ExitStack() as astack:
            ptp = astack.enter_context(tc.tile_pool(name="pt", bufs=8))
            stg = astack.enter_context(tc.tile_pool(name="stg", bufs=6))
            nrm = astack.enter_context(tc.tile_pool(name="nrm", bufs=6))
            wop = astack.enter_context(tc.tile_pool(name="wo", bufs=NM))
            oap = astack.enter_context(tc.tile_pool(name="oacc", bufs=32))
            sp = astack.enter_context(
                tc.tile_pool(name="sps", bufs=2, space="PSUM"))
            pvp = astack.enter_context(
                tc.tile_pool(name="pv", bufs=2, space="PSUM"))
            opp = astack.enter_context(
                tc.tile_pool(name="op", bufs=2, space="PSUM"))

            wo_tiles = []
            for m in range(NM):
                w = wop.tile([128, D], OUT_DT, tag="wo", name=f"wo{m}")
                nc.sync.dma_start(out=w, in_=Wo[m * 128:(m + 1) * 128, :])
                wo_tiles.append(w)
            out_acc = [[None] * 2 for _ in range(NKT)]

            def outproj_piece(hp, st):
                for oc in range(2):
                    ps = opp.tile([128, 512], F32, tag="op",
                                  name=f"ops{hp}_{st}_{oc}")
                    nc.tensor.matmul(
                        ps,
                        attn_outT[hp][:, st * 128:(st + 1) * 128],
                        wo_tiles[hp][:, oc * 512:(oc + 1) * 512],
                        start=True, stop=True)
                    if hp == 0:
                        oa = oap.tile([128, 512], F32, tag="oacc",
                                      name=f"oacc{st}_{oc}")
                        out_acc[st][oc] = oa
                        nc.vector.tensor_copy(oa, ps)
                    else:
                        oa = out_acc[st][oc]
                        nc.vector.tensor_add(oa, oa, ps)
                    if hp == NM - 1:
                        nc.sync.dma_start(
                            out=OP[st * 128:(st + 1) * 128,
                                   oc * 512:(oc + 1) * 512],
                            in_=oa)

            # deferred tail/outproj pieces, drained at fixed kt slots of
            # the NEXT block so slow DVE work never head-of-line blocks PE
            deferred = []

            def make_tail_pieces(hp, qc, pvA, pvB):
                q0 = qc * 512
                sts = [None, None]
                rzs = [None, None]
                bcs = [None, None]

                def stage(hh):
                    acc = pvA if hh == 0 else pvB
                    st_t = stg.tile([DH + 1, 512], F32R, tag="stg",
                                    name=f"stg{hp}_{qc}_{hh}")
                    with nc.allow_low_precision(reason="stage"):
                        nc.vector.tensor_copy(st_t, acc)
                    sts[hh] = st_t

                def recip(hh):
                    rz = nrm.tile([1, 512], F32R, tag="rz",
                                  name=f"rz{hp}_{qc}_{hh}")
                    with nc.allow_low_precision(reason="recipZ"):
                        nc.vector.reciprocal(out=rz, in_=sts[hh][DH:DH + 1, :])
                    rzs[hh] = rz

                def bcast(hh):
                    bc_ps = opp.tile([DH, 512], F32, tag="op",
                                     name=f"bcp{hp}_{qc}_{hh}")
                    nc.tensor.matmul(bc_ps, ones_row, rzs[hh],
                                     start=True, stop=True)
                    bc = nrm.tile([DH, 512], F32R, tag="bc",
                                  name=f"bc{hp}_{qc}_{hh}")
                    with nc.allow_low_precision(reason="bc"):
                        nc.vector.tensor_copy(bc, bc_ps)
                    bcs[hh] = bc

                def mul(hh):
                    dlo = hh * DH
                    with nc.allow_low_precision(reason="attn_outT"):
                        nc.vector.tensor_mul(
                            attn_outT[hp][dlo:dlo + DH, q0:q0 + 512],
                            sts[hh][0:DH, :], bcs[hh])

                # stages run NOW (free the PSUM accumulators quickly)
                stage(0)
                stage(1)
                return [
                    lambda: recip(0),
                    lambda: recip(1),
                    lambda: bcast(0),
                    lambda: bcast(1),
                    lambda: mul(0),
                    lambda: mul(1),
                    lambda: outproj_piece(hp, qc * 4 + 0),
                    lambda: outproj_piece(hp, qc * 4 + 1),
                    lambda: outproj_piece(hp, qc * 4 + 2),
                    lambda: outproj_piece(hp, qc * 4 + 3),
                ]

            # kt slots at which deferred pieces fire (10 pieces)
            SLOTS = {0: 0, 1: 1, 3: 2, 4: 3, 6: 4, 7: 5,
                     9: 6, 11: 7, 13: 8, 15: 9}

            for hp in range(NM):
                for qc in range(4):
                    q0 = qc * 512
                    pvA = pvp.tile([DH + 1, 512], F32, tag="pv",
                                   name=f"pvA{hp}_{qc}")
                    pvB = pvp.tile([DH + 1, 512], F32, tag="pv",
                                   name=f"pvB{hp}_{qc}")
                    for kt in range(NKT):
                        sps = sp.tile([128, 1024], F32, tag="sps")
                        for hh in range(2):
                            dlo = hh * DH
                            nc.tensor.matmul(
                                sps[:, hh * 512:(hh + 1) * 512],
                                kT[hp][dlo:dlo + DH,
                                       kt * 128:(kt + 1) * 128],
                                qT[hp][dlo:dlo + DH, q0:q0 + 512],
                                start=True, stop=True)
                        pt = ptp.tile([128, 1024], PV_DT, tag="pt")
                        nc.scalar.activation(
                            out=pt, in_=sps,
                            func=mybir.ActivationFunctionType.Exp,
                            scale=float(SCALE))
                        nc.tensor.matmul(
                            pvA, vhat[kt][:, 2 * hp, :], pt[:, 0:512],
                            start=(kt == 0), stop=(kt == NKT - 1))
                        nc.tensor.matmul(
                            pvB, vhat[kt][:, 2 * hp + 1, :], pt[:, 512:1024],
                            start=(kt == 0), stop=(kt == NKT - 1))
                        if kt in SLOTS and deferred:
                            deferred[SLOTS[kt]]()
                    deferred = make_tail_pieces(hp, qc, pvA, pvB)

            # flush the last block's pieces
            for piece in deferred:
                piece()
            return out_acc[NKT - 1][1]


_NC_CACHE = None
_last_in_maps = None


def _get_nc():
    global _NC_CACHE
    if _NC_CACHE is None:
        _NC_CACHE = build()
    return _NC_CACHE


def kernel(Q, K, V, W_Q, b_Q, W_K, b_K, W_V, b_V, W_O, b_O):
    global _last_in_maps
    Q = np.asarray(Q, dtype=np.float32)
    K = np.asarray(K, dtype=np.float32)
    V = np.asarray(V, dtype=np.float32)
    nc = _get_nc()

    XqTs = [prep(Q[b].T, PROJ_DT) for b in range(B)]
    XkTs = [prep(K[b].T, PROJ_DT) for b in range(B)]
    XvTs = [prep(V[b].T, PROJ_DT) for b in range(B)]
    Wqs = [prep(np.asarray(W_Q)[:, hg * C:(hg + 1) * C], PROJ_DT)
           for hg in range(2)]
    Wks = [prep(np.asarray(W_K)[:, hg * C:(hg + 1) * C], PROJ_DT)
           for hg in range(2)]
    Wvs = [prep(np.asarray(W_V)[:, hg * C:(hg + 1) * C], PROJ_DT)
           for hg in range(2)]
    Wos = [prep(np.asarray(W_O)[hg * C:(hg + 1) * C, :], OUT_DT)
           for hg in range(2)]
    bqs = [np.ascontiguousarray(np.asarray(b_Q, dtype=np.float32)[hg * C:(hg + 1) * C])
           for hg in range(2)]
    bks = [np.ascontiguousarray(np.asarray(b_K, dtype=np.float32)[hg * C:(hg + 1) * C])
           for hg in range(2)]
    bvs = [np.ascontiguousarray(np.asarray(b_V, dtype=np.float32)[hg * C:(hg + 1) * C])
           for hg in range(2)]

    in_maps = []
    for c in range(N_CORES):
        b, hg = c // 2, c % 2
        in_maps.append({
            "XqT": XqTs[b], "XkT": XkTs[b], "XvT": XvTs[b],
            "Wq": Wqs[hg], "Wk": Wks[hg], "Wv": Wvs[hg], "Wo": Wos[hg],
            "bq": bqs[hg], "bk": bks[hg], "bv": bvs[hg],
        })
    _last_in_maps = in_maps
    res = run_bass_kernel_spmd(nc, in_maps, list(range(N_CORES)))
    out = np.empty((B, S, D), dtype=np.float32)
    bO = np.asarray(b_O, dtype=np.float32)
    for b in range(B):
        out[b] = res.results[2 * b]["OP"] + res.results[2 * b + 1]["OP"] + bO
    return out



# revision 6
# speedup vs baseline: 16.4836x; 1.9658x over previous
"""Multi-head attention (B=4, S=2048, D=1024, H=16) on 8 trn2 NeuronCores.

Sharding: core c -> (batch b = c//2, head-group hg = c%2 of 8 heads).
Host sums the two partial output projections per batch and adds b_O.

v2 design (vs baseline): PV matmul in natural [q, d+1] layout (65-cycle
matmuls instead of 512 at half occupancy), per-partition softmax
normalization (Z rides along as vhat's 65th column), attn_out transposed
via DMA XBAR (14ns/16x128-tile) instead of PE broadcast matmuls, output
projection K-accumulated across head pairs in PSUM (one copy per tile
instead of 4 copy/adds), projection epilogues on DVE, exp exclusively on
Act, projections software-pipelined into the first attention blocks.

PSUM budget (8 banks): sps 2x[128,1024]=4, acc 2x[128,260]=2 (bufs=1),
outproj 2x[128,512]=2.
"""
import sys

if '/opt/trn_rl_repo' not in sys.path:
    sys.path.insert(0, '/opt/trn_rl_repo')

import ml_dtypes
import numpy as np

import concourse.bass as bass
import concourse.tile as tile
from concourse import bacc, mybir
from concourse.bass_utils import run_bass_kernel_spmd

N_CORES = 8
B, S, D = 4, 2048, 1024
H = 16
DH = 64                 # head dim
HC = 8                  # heads per core
C = HC * DH             # per-core projection width = 512
F32 = mybir.dt.float32
F32R = mybir.dt.float32r
BF16 = mybir.dt.bfloat16

NKT = S // 128          # 16 s-tiles of 128
NM = C // 128           # 4 c-tiles (head pairs)
NDK = D // 128          # 8 contraction tiles for projections
SCALE = 1.0 / np.sqrt(DH)

PROJ_DT = BF16
QK_DT = BF16
PV_DT = BF16
OUT_DT = BF16


def round_fp32r(x):
    b = np.ascontiguousarray(x, dtype=np.float32).view(np.uint32)
    b = (b + 0x800) & np.uint32(0xFFFFF000)
    return b.view(np.float32)


def prep(x, dt):
    if dt == BF16:
        return np.ascontiguousarray(x).astype(ml_dtypes.bfloat16)
    return round_fp32r(x)


def build(repeat=1, dbg=False):
    nc = bacc.Bacc("TRN2", target_bir_lowering=False, debug=False,
                   num_devices=N_CORES)
    XqT = nc.dram_tensor("XqT", [D, S], PROJ_DT, kind="ExternalInput").ap()
    XkT = nc.dram_tensor("XkT", [D, S], PROJ_DT, kind="ExternalInput").ap()
    XvT = nc.dram_tensor("XvT", [D, S], PROJ_DT, kind="ExternalInput").ap()
    Wq = nc.dram_tensor("Wq", [D, C], PROJ_DT, kind="ExternalInput").ap()
    Wk = nc.dram_tensor("Wk", [D, C], PROJ_DT, kind="ExternalInput").ap()
    Wv = nc.dram_tensor("Wv", [D, C], PROJ_DT, kind="ExternalInput").ap()
    Wo = nc.dram_tensor("Wo", [C, D], OUT_DT, kind="ExternalInput").ap()
    bq = nc.dram_tensor("bq", [C], F32, kind="ExternalInput").ap()
    bk = nc.dram_tensor("bk", [C], F32, kind="ExternalInput").ap()
    bv = nc.dram_tensor("bv", [C], F32, kind="ExternalInput").ap()
    OP = nc.dram_tensor("OP", [S, D], OUT_DT, kind="ExternalOutput").ap()
    dbg_t = None
    if dbg:
        dbg_t = {
            'qT0': nc.dram_tensor("DBG_qT0", [128, S], QK_DT, kind="ExternalOutput").ap(),
            'kT0': nc.dram_tensor("DBG_kT0", [128, S], QK_DT, kind="ExternalOutput").ap(),
            'vh0': nc.dram_tensor("DBG_vh0", [128, HC, DH + 1], PV_DT, kind="ExternalOutput").ap(),
            'pt0': nc.dram_tensor("DBG_pt0", [128, 1024], PV_DT, kind="ExternalOutput").ap(),
            'st0': nc.dram_tensor("DBG_st0", [128, 2, 4, DH + 1], OUT_DT, kind="ExternalOutput").ap(),
            'nat0': nc.dram_tensor("DBG_nat0", [128, 128], OUT_DT, kind="ExternalOutput").ap(),
            'aot0': nc.dram_tensor("DBG_aot0", [128, 128], OUT_DT, kind="ExternalOutput").ap(),
        }

    SCR = None
    if repeat > 1:
        SCR = nc.dram_tensor("SCR", [1, 8], OUT_DT, kind="Internal").ap()
    with tile.TileContext(nc) as tc:
        token = None
        for _rep in range(repeat):
            token = _build_body(nc, tc, XqT, XkT, XvT, Wq, Wk, Wv, Wo,
                                bq, bk, bv, OP, dbg_t, gate=token, SCR=SCR)
    nc.compile()
    return nc


def _build_body(nc, tc, XqT, XkT, XvT, Wq, Wk, Wv, Wo, bq, bk, bv, OP,
                dbg_t=None, gate=None, SCR=None):
    from contextlib import ExitStack
    with ExitStack() as stack:
        consts = stack.enter_context(tc.tile_pool(name="consts", bufs=1))
        qkp = stack.enter_context(tc.tile_pool(name="qk", bufs=2 * NM))
        vhp = stack.enter_context(tc.tile_pool(name="vh", bufs=NKT))
        wop = stack.enter_context(tc.tile_pool(name="wo", bufs=NM))
        xwp = stack.enter_context(tc.tile_pool(name="xw", bufs=1))
        ptp = stack.enter_context(tc.tile_pool(name="pt", bufs=3))
        stgp = stack.enter_context(tc.tile_pool(name="stg", bufs=2))
        rzp = stack.enter_context(tc.tile_pool(name="rz", bufs=2))
        natp = stack.enter_context(tc.tile_pool(name="nat", bufs=3))
        aotp = stack.enter_context(tc.tile_pool(name="aot", bufs=64))
        oap = stack.enter_context(tc.tile_pool(name="oa", bufs=4))
        spp = stack.enter_context(
            tc.tile_pool(name="sps", bufs=2, space="PSUM"))
        accp = stack.enter_context(
            tc.tile_pool(name="acc", bufs=1, space="PSUM"))
        opp = stack.enter_context(
            tc.tile_pool(name="op", bufs=2, space="PSUM"))

        # serialize repeat bodies: a gate DMA on the SP queue reading the
        # previous body's last output tile; FIFO queue order then holds all
        # of this body's input DMAs behind it (timing-harness only)
        if gate is not None:
            nc.sync.dma_start(out=SCR, in_=gate[0:1, 0:8])
            nc.scalar.dma_start(out=SCR, in_=gate[0:1, 0:8])

        # ---- constants ----
        ones_f32 = consts.tile([128, 1], F32)
        nc.vector.memset(ones_f32, 1.0)
        bias_t = consts.tile([128, 2 * NM], F32)
        for i, b_ in enumerate((bq, bk)):
            nc.sync.dma_start(
                out=bias_t[:, i * NM:(i + 1) * NM],
                in_=b_.rearrange("(m p) -> p m", p=128))
        bvb = consts.tile([128, C], F32)
        nc.gpsimd.dma_start(
            out=bvb,
            in_=bass.AP(tensor=bv.tensor, offset=0, ap=[[0, 128], [1, C]]))

        # ---- input DMAs, ordered for earliest attention start ----
        # Each tensor is ONE SBUF tile [128, 8, width] (kt on the middle
        # axis); X streams in four 512-col chunks (one DMA op each, all 8
        # kt sub-tiles per op), interleaved k/q/v so early kt scores and
        # early st vhat unblock while later chunks stream.
        def declx(name, width):
            return xwp.tile([128, NDK, width], PROJ_DT, tag=name,
                            name=name)

        def loadw(t, src, width, eng=None):
            (eng or nc.sync).dma_start(
                out=t, in_=src.rearrange("(t p) c -> p t c", p=128))

        def loadx_sc(t, src, sc, eng=None):
            c0 = sc * 512
            (eng or nc.sync).dma_start(
                out=t[:, :, c0:c0 + 512],
                in_=src[:, c0:c0 + 512].rearrange("(t p) c -> p t c", p=128))

        xk_t = declx("xk", S)
        xq_t = declx("xq", S)
        xv_t = declx("xv", S)
        wk_t = declx("wk", C)
        wq_t = declx("wq", C)
        wv_t = declx("wv", C)
        # alternate queues: SP and Act HWDGE rings load in parallel on HW
        loadw(wk_t, Wk, C)
        loadx_sc(xk_t, XkT, 0, eng=nc.scalar)
        loadw(wq_t, Wq, C)
        loadx_sc(xq_t, XqT, 0, eng=nc.scalar)
        loadw(wv_t, Wv, C)
        loadx_sc(xv_t, XvT, 0, eng=nc.scalar)
        for sc in range(1, 4):
            loadx_sc(xk_t, XkT, sc)
            loadx_sc(xv_t, XvT, sc, eng=nc.scalar)
            loadx_sc(xq_t, XqT, sc)
        wo_t = []
        for m in range(NM):
            w = wop.tile([128, D], OUT_DT, tag="wo", name=f"wo{m}")
            nc.sync.dma_start(out=w, in_=Wo[m * 128:(m + 1) * 128, :])
            wo_t.append(w)

        # ---- projection targets ----
        qT = [qkp.tile([128, S], QK_DT, tag="qk", name=f"qT{m}")
              for m in range(NM)]
        kT = [qkp.tile([128, S], QK_DT, tag="qk", name=f"kT{m}")
              for m in range(NM)]
        vhat = [None] * NKT

        def proj_qk_piece(w_t, x_t, dst, m, sc, bcol):
            # dst[m][:, sc*512:(sc+1)*512] = (W^T X)[...] + bias, sc in 0..3
            ps = spp.tile([128, 512], F32, tag="sps", name=f"pj{m}_{sc}")
            c0 = sc * 512
            for kt in range(NDK):
                nc.tensor.matmul(
                    ps,
                    w_t[:, kt, m * 128:(m + 1) * 128],
                    x_t[:, kt, c0:c0 + 512],
                    start=(kt == 0), stop=(kt == NDK - 1))
            with nc.allow_low_precision(reason="proj epilogue"):
                nc.vector.tensor_scalar_add(
                    dst[m][:, c0:c0 + 512], ps,
                    bias_t[:, bcol + m:bcol + m + 1])

        def proj_qk_streamed(w_t, x_t, dst, m, sc, bcol):
            # same computation, split into 8 single-matmul sub-pieces (one
            # per kt slot) + an epilogue; accumulates through the outproj
            # PSUM pool (temporally disjoint from outproj pieces)
            c0 = sc * 512
            state = {}

            def mk(i):
                def f():
                    if i == 0:
                        state['ps'] = opp.tile([128, 512], F32, tag="op",
                                               name=f"pjs{m}_{sc}")
                    nc.tensor.matmul(
                        state['ps'],
                        w_t[:, i, m * 128:(m + 1) * 128],
                        x_t[:, i, c0:c0 + 512],
                        start=(i == 0), stop=(i == NDK - 1))
                return f

            def epi():
                with nc.allow_low_precision(reason="proj epilogue"):
                    nc.vector.tensor_scalar_add(
                        dst[m][:, c0:c0 + 512], state['ps'],
                        bias_t[:, bcol + m:bcol + m + 1])

            return [mk(i) for i in range(NDK)] + [epi]

        def proj_v_piece(st2):
            # vhat[2*st2], vhat[2*st2+1]
            ps = spp.tile([128, 1024], F32, tag="sps", name=f"vps{st2}")
            for half in range(2):
                st = st2 * 2 + half
                for kt in range(NDK):
                    nc.tensor.matmul(
                        ps[:, half * 512:(half + 1) * 512],
                        xv_t[:, kt, st * 128:(st + 1) * 128],
                        wv_t[:, kt, :],
                        start=(kt == 0), stop=(kt == NDK - 1))
            for half in range(2):
                st = st2 * 2 + half
                vh = vhp.tile([128, HC, DH + 1], PV_DT, tag="vh",
                              name=f"vhat{st}")
                with nc.allow_low_precision(reason="v epilogue"):
                    nc.vector.tensor_add(
                        vh[:, :, 0:DH],
                        ps[:, half * 512:(half + 1) * 512].rearrange(
                            "p (h d) -> p h d", h=HC),
                        bvb.rearrange("p (h d) -> p h d", h=HC))
                    nc.vector.tensor_copy(
                        vh[:, :, DH], ones_f32.broadcast_to((128, HC)))
                vhat[st] = vh

        # ---- deferred piece machinery ----
        pending = []

        def drain(k):
            for _ in range(min(k, len(pending))):
                pending.pop(0)()

        # prefix: kT0 cols 0-511, qT0 cols 0-511, vhat 0..3
        proj_qk_piece(wk_t, xk_t, kT, 0, 0, NM)
        proj_qk_piece(wq_t, xq_t, qT, 0, 0, 0)
        proj_v_piece(0)
        proj_v_piece(1)

        # static just-in-time schedule for remaining projection pieces:
        # (block, kt) -> pieces, tracking both consumer deadlines and the
        # interleaved DMA arrival order.
        sched = {}

        def at(b, kt, fn):
            sched.setdefault((b, kt), []).append(fn)

        def qk(which, m, sc):
            if which == 'k':
                return lambda: proj_qk_piece(wk_t, xk_t, kT, m, sc, NM)
            return lambda: proj_qk_piece(wq_t, xq_t, qT, m, sc, 0)

        # block 0: rest of kT0 (contiguous, needed by this very block's
        # kt loop), vhat 4..15, qT0 for qc1
        at(0, 1, qk('k', 0, 1))
        at(0, 1, lambda: proj_v_piece(2))
        at(0, 3, qk('k', 0, 2))
        at(0, 3, lambda: proj_v_piece(3))
        at(0, 5, qk('k', 0, 3))
        at(0, 5, lambda: proj_v_piece(4))
        at(0, 7, qk('q', 0, 1))
        at(0, 7, lambda: proj_v_piece(5))
        at(0, 9, lambda: proj_v_piece(6))
        at(0, 11, lambda: proj_v_piece(7))
        at(1, 1, qk('q', 0, 2))
        at(1, 5, qk('q', 0, 3))
        # kT[m]/qT[m] for m=1..3: streamed as pairs of single-matmul
        # sub-pieces, 2 inserted per kt (fits Act's per-kt slack), riding
        # the outproj PSUM pool which is idle until the hp=3 sweep
        streamed = []
        for m in range(1, NM):
            for sc in range(4):
                streamed.append(('k', m, sc))
            for sc in range(4):
                streamed.append(('q', m, sc))

        L0 = 1 * 16 + 8      # linear kt index: start at block 1, kt 8
        for pair in range(0, len(streamed), 2):
            base = L0 + (pair // 2) * 12
            for j in (0, 1):
                if pair + j >= len(streamed):
                    break
                which, m, sc = streamed[pair + j]
                w_t, x_t, dst, bcol = (
                    (wk_t, xk_t, kT, NM) if which == 'k'
                    else (wq_t, xq_t, qT, 0))
                subs = proj_qk_streamed(w_t, x_t, dst, m, sc, bcol)
                for i, fn in enumerate(subs):
                    Lx = base + i + j  # stagger the pair by one kt
                    at(Lx // 16, Lx % 16, fn)

        # ---- attention (hp outer: Act starts after just kT0/qT0) ----
        aoT = {}
        last_oa = [None]

        def outproj_piece(qc, qs, oc):
            ps = opp.tile([128, 512], F32, tag="op", name=f"ops{qc}_{qs}_{oc}")
            for hp in range(NM):
                nc.tensor.matmul(
                    ps,
                    aoT[(hp, qc, qs)],
                    wo_t[hp][:, oc * 512:(oc + 1) * 512],
                    start=(hp == 0), stop=(hp == NM - 1))
            oa = oap.tile([128, 512], OUT_DT, tag="oa", name=f"oa{qc}_{qs}_{oc}")
            with nc.allow_low_precision(reason="OP bf16"):
                nc.vector.tensor_copy(oa, ps)
            last_oa[0] = oa
            st = qc * 4 + qs
            nc.scalar.dma_start(
                out=OP[st * 128:(st + 1) * 128, oc * 512:(oc + 1) * 512],
                in_=oa)

        def make_tail(hp, qc, acc_a, acc_b):
            # stage on DVE immediately: frees acc (bufs=1) for the next
            # block; the window until that block's first PV is ~1.5us
            st_t = stgp.tile([128, 2, 4, DH + 1], OUT_DT, tag="stg",
                             name=f"stg{hp}_{qc}")
            with nc.allow_low_precision(reason="stage"):
                nc.vector.tensor_copy(st_t[:, 0], acc_a)
                nc.vector.tensor_copy(st_t[:, 1], acc_b)
            if dbg_t is not None and hp == 0 and qc == 0:
                nc.sync.dma_start(out=dbg_t['st0'], in_=st_t)
            rz = rzp.tile([128, 2, 4], F32, tag="rz", name=f"rz{hp}_{qc}")

            def recip():
                with nc.allow_low_precision(reason="recipZ"):
                    nc.vector.reciprocal(out=rz, in_=st_t[:, :, :, DH])

            def norm_tp(qs):
                nat = natp.tile([128, 128], OUT_DT, tag="nat",
                                name=f"nat{hp}_{qc}_{qs}")
                with nc.allow_low_precision(reason="normalize"):
                    nc.vector.tensor_scalar_mul(
                        nat[:, 0:DH], st_t[:, 0, qs, 0:DH],
                        rz[:, 0, qs:qs + 1])
                    nc.vector.tensor_scalar_mul(
                        nat[:, DH:128], st_t[:, 1, qs, 0:DH],
                        rz[:, 1, qs:qs + 1])
                ao = aotp.tile([128, 128], OUT_DT, tag="aot",
                               name=f"aoT{hp}_{qc}_{qs}")
                nc.scalar.dma_start_transpose(out=ao, in_=nat)
                aoT[(hp, qc, qs)] = ao
                if dbg_t is not None and hp == 0 and qc == 0 and qs == 0:
                    nc.sync.dma_start(out=dbg_t['nat0'], in_=nat)
                    nc.sync.dma_start(out=dbg_t['aot0'], in_=ao)

            return [recip] + [lambda qs=qs: norm_tp(qs) for qs in range(4)]

        if dbg_t is not None:
            pending.append(lambda: nc.sync.dma_start(out=dbg_t['qT0'], in_=qT[0]))
            pending.append(lambda: nc.sync.dma_start(out=dbg_t['kT0'], in_=kT[0]))
            pending.append(lambda: nc.sync.dma_start(out=dbg_t['vh0'], in_=vhat[0]))
        SLOTS = (1, 3, 5, 7, 9, 11, 13, 15)
        for hp in range(NM):
            for qc in range(4):
                acc_a = accp.tile([128, 4, DH + 1], F32, tag="acc_a",
                                  name=f"acca{hp}_{qc}")
                acc_b = accp.tile([128, 4, DH + 1], F32, tag="acc_b",
                                  name=f"accb{hp}_{qc}")
                for kt in range(NKT):
                    sps = spp.tile([128, 1024], F32, tag="sps",
                                   name=f"sps{hp}_{qc}_{kt}")
                    for hh in range(2):
                        dlo = hh * DH
                        nc.tensor.matmul(
                            sps[:, hh * 512:(hh + 1) * 512],
                            kT[hp][dlo:dlo + DH, kt * 128:(kt + 1) * 128],
                            qT[hp][dlo:dlo + DH, qc * 512:(qc + 1) * 512],
                            start=True, stop=True)
                    pt = ptp.tile([128, 1024], PV_DT, tag="pt",
                                  name=f"pt{hp}_{qc}_{kt}")
                    nc.scalar.activation(
                        out=pt, in_=sps,
                        func=mybir.ActivationFunctionType.Exp,
                        scale=float(SCALE))
                    if dbg_t is not None and hp == 0 and qc == 0 and kt == 0:
                        nc.sync.dma_start(out=dbg_t['pt0'], in_=pt)
                    for h in range(2):
                        acc = acc_a if h == 0 else acc_b
                        for qs in range(4):
                            # one accumulation group per PSUM bank: start
                            # zeroes the whole 2KB zero-region, so only the
                            # first matmul starts and only the last stops
                            nc.tensor.matmul(
                                acc[:, qs, :],
                                pt[:, h * 512 + qs * 128:h * 512 + (qs + 1) * 128],
                                vhat[kt][:, 2 * hp + h, :],
                                start=(kt == 0 and qs == 0),
                                stop=(kt == NKT - 1 and qs == 3))
                    blk = hp * 4 + qc
                    for fn in sched.pop((blk, kt), ()):
                        fn()
                    if kt in SLOTS:
                        drain(2)
                pending = make_tail(hp, qc, acc_a, acc_b) + pending
                if hp == NM - 1:
                    for qs in range(4):
                        for oc in range(2):
                            pending.append(
                                lambda qc=qc, qs=qs, oc=oc:
                                outproj_piece(qc, qs, oc))
        drain(len(pending))
        return last_oa[0]


_NC_CACHE = None
_last_in_maps = None


def _get_nc():
    global _NC_CACHE
    if _NC_CACHE is None:
        _NC_CACHE = build()
    return _NC_CACHE


def kernel(Q, K, V, W_Q, b_Q, W_K, b_K, W_V, b_V, W_O, b_O):
    global _last_in_maps
    Q = np.asarray(Q, dtype=np.float32)
    K = np.asarray(K, dtype=np.float32)
    V = np.asarray(V, dtype=np.float32)
    nc = _get_nc()

    XqTs = [prep(Q[b].T, PROJ_DT) for b in range(B)]
    XkTs = [prep(K[b].T, PROJ_DT) for b in range(B)]
    XvTs = [prep(V[b].T, PROJ_DT) for b in range(B)]
    Wqs = [prep(np.asarray(W_Q)[:, hg * C:(hg + 1) * C], PROJ_DT)
           for hg in range(2)]
    Wks = [prep(np.asarray(W_K)[:, hg * C:(hg + 1) * C], PROJ_DT)
           for hg in range(2)]
    Wvs = [prep(np.asarray(W_V)[:, hg * C:(hg + 1) * C], PROJ_DT)
           for hg in range(2)]
    Wos = [prep(np.asarray(W_O)[hg * C:(hg + 1) * C, :], OUT_DT)
           for hg in range(2)]
    bqs = [np.ascontiguousarray(np.asarray(b_Q, dtype=np.float32)[hg * C:(hg + 1) * C])
           for hg in range(2)]
    bks = [np.ascontiguousarray(np.asarray(b_K, dtype=np.float32)[hg * C:(hg + 1) * C])
           for hg in range(2)]
    bvs = [np.ascontiguousarray(np.asarray(b_V, dtype=np.float32)[hg * C:(hg + 1) * C])
           for hg in range(2)]

    in_maps = []
    for c in range(N_CORES):
        b, hg = c // 2, c % 2
        in_maps.append({
            "XqT": XqTs[b], "XkT": XkTs[b], "XvT": XvTs[b],
            "Wq": Wqs[hg], "Wk": Wks[hg], "Wv": Wvs[hg], "Wo": Wos[hg],
            "bq": bqs[hg], "bk": bks[hg], "bv": bvs[hg],
        })
    _last_in_maps = in_maps
    res = run_bass_kernel_spmd(nc, in_maps, list(range(N_CORES)))
    out = np.empty((B, S, D), dtype=np.float32)
    bO = np.asarray(b_O, dtype=np.float32)
    for b in range(B):
        out[b] = (res.results[2 * b]["OP"].astype(np.float32)
                  + res.results[2 * b + 1]["OP"].astype(np.float32) + bO)
    return out
